# revision 56
# baseline (speedup 1.0000x reference)
"""Trainium2 Bass kernel for nn_BlockRecurrentSwinIRBlock (self-contained).

kernel(**inputs) takes the FULL unsharded inputs (B=2048 windows, 64 tokens,
C=256) and returns (output_x, state_out), each (2048, 64, 256) fp32.

Strategy: data-parallel over windows across 8 NeuronCores (256 windows/core).
All matmuls bf16 with fp32 PSUM accumulation.

v2 changes over v1 (1.71ms):
  - software pipeline restructured: out-projections lag their window-pair by
    two steps in the Tensor queue, so the Vector-side normalize chain
    (reciprocal + mul) overlaps the next pair's QK/AV instead of stalling PE.
  - j-major E layout -> exp activations write contiguous destinations.
  - d-banks use the same (s,hi,t,n) column order as o-banks and mirror keeps
    pp innermost -> normalization is 2 recips + 2 muls per pair (was 2+4).
  - elementwise work split across Scalar (exp, vproj cast), Vector (recip,
    bias-mul, one norm-mul), GpSimd (one norm-mul, proj1 copies).
  - out-projection results DMA'd straight from PSUM (no SBUF staging).
  - vproj PSUM pairs two window-pairs per bank -> half the casts.
"""
import sys
import numpy as np

for _p in ("/opt/trn_rl_repo", "/opt/trn_rl_repo/concourse"):
    if _p not in sys.path:
        sys.path.insert(0, _p)

import concourse.bass as bass
import concourse.tile as tile
from concourse import bacc, mybir
from concourse.bass_utils import run_bass_kernel_spmd
from contextlib import ExitStack

N_CORES = 8
NTOK = 64
DIM = 256
B_TOTAL = 2048
W_CORE = B_TOTAL // N_CORES
UNROLL = 32

_cache = {}

BF16 = mybir.dt.bfloat16
FP8 = mybir.dt.float8e4
F32 = mybir.dt.float32
AF = mybir.ActivationFunctionType
DR = mybir.MatmulPerfMode.DoubleRow
CH = 4          # window-pairs per chunk
# fp8 prescales: k x64, qv/qh x512, v x64 (powers of two; folded back out via
# the exp scale argument and the out-projection weights)
K_SC = 1.0
Q_SC = 1.0
V_SC = 1.0
S_SCALE = 1.0


def build_kernel(nc, tc, W, unroll=UNROLL):
    WP = W // 2
    T = W * 64
    NCH = WP // CH
    assert WP % CH == 0

    xt_s = nc.dram_tensor("xt_s", [WP, 2, 128, 128], BF16, kind="ExternalInput").ap()
    xt_e = nc.dram_tensor("xt_e", [WP, 2, 128, 128], BF16, kind="ExternalInput").ap()
    wkq_s = nc.dram_tensor("wkq_s", [2, 128, 768], BF16, kind="ExternalInput").ap()
    wkq_e = nc.dram_tensor("wkq_e", [2, 128, 768], BF16, kind="ExternalInput").ap()
    wv_s = nc.dram_tensor("wv_s", [2, 128, 256], BF16, kind="ExternalInput").ap()
    wv_e = nc.dram_tensor("wv_e", [2, 128, 256], BF16, kind="ExternalInput").ap()
    wpv = nc.dram_tensor("wpv", [4, 128, 256], BF16, kind="ExternalInput").ap()
    wph = nc.dram_tensor("wph", [4, 128, 256], BF16, kind="ExternalInput").ap()
    # combined exp(bias) master, j-major: col 1024*j? -> see eb layout below:
    # col = j*512 + s*256 + hi*128 + t*64 + n   (rows: p*64 + ktok)
    ebc = nc.dram_tensor("ebc", [128, 2048], BF16, kind="ExternalInput").ap()

    out_x = nc.dram_tensor("out_x", [T, 256], F32, kind="ExternalOutput").ap()
    out_h = nc.dram_tensor("out_h", [T, 256], F32, kind="ExternalOutput").ap()

    xt_d = {0: xt_s, 1: xt_e}
    wkq_d = {0: wkq_s, 1: wkq_e}
    wv_d = {0: wv_s, 1: wv_e}

    with ExitStack() as ctx:
        wpool = ctx.enter_context(tc.tile_pool(name="weights", bufs=1))
        sb = ctx.enter_context(tc.tile_pool(name="sb", bufs=3))
        pp = ctx.enter_context(tc.tile_pool(name="pp", bufs=4, space="PSUM"))

        wkq = {}
        wv = {}
        for s in (0, 1):
            t_ = wpool.tile([128, 2 * 768], BF16, tag=f"wkq{s}", name=f"wkq{s}t")
            for kk in (0, 1):
                nc.sync.dma_start(t_[:, kk * 768:(kk + 1) * 768], wkq_d[s][kk])
            wkq[s] = t_
            tv = wpool.tile([128, 2 * 256], BF16, tag=f"wv{s}", name=f"wv{s}t")
            for kk in (0, 1):
                nc.sync.dma_start(tv[:, kk * 256:(kk + 1) * 256], wv_d[s][kk])
            wv[s] = tv
        ebt = wpool.tile([128, 2048], BF16, tag="ebc", name="ebt")
        wp_t = {}
        for nm in ("x", "h"):
            wp_t[nm] = wpool.tile([128, 4 * 256], BF16, tag=f"wp{nm}",
                                  name=f"wp{nm}t")
        ones32 = wpool.tile([128, 32], BF16, tag="ones", name="ones32")
        nc.vector.memset(ones32[:], 1.0)

        _lw = {"done": False}

        def dma_late_weights():
            # ebt / out-proj weights aren't needed until ~15us in; keep them
            # off the startup DMA critical path.
            if _lw["done"]:
                return
            _lw["done"] = True
            nc.sync.dma_start(ebt[:], ebc[:])
            for nm, apw in (("x", wpv), ("h", wph)):
                for z in range(4):
                    nc.sync.dma_start(
                        wp_t[nm][:, z * 256:(z + 1) * 256], apw[z])

        def dma_xt(ci):
            # one DMA per side: [CH, 2, 128, 128] -> [128, (kk, w4, c)]
            XT = {}
            for s in (0, 1):
                t_ = sb.tile([128, 1024], BF16, tag=f"xt{s}", name=f"xt{s}", bufs=2)
                if isinstance(ci, int):
                    src = xt_d[s][ci * CH:(ci + 1) * CH]
                else:
                    src = xt_d[s][bass.ds(ci * CH, CH)]
                for kk in (0, 1):
                    nc.sync.dma_start(
                        t_[:, kk * 512:(kk + 1) * 512].rearrange(
                            "p (w4 c) -> p w4 c", w4=CH),
                        src[:, kk].rearrange("w4 p c -> p w4 c"))
                XT[s] = t_
            return XT

        def alloc_proj():
            ykc = {}
            yqc = {}
            for s in (0, 1):
                ykc[s] = sb.tile([128, 1024], BF16, tag=f"yk{s}", name=f"yk{s}",
                                 bufs=2)
                yqc[s] = sb.tile([128, 2048], BF16, tag=f"yq{s}", name=f"yq{s}",
                                 bufs=2)
            return ykc, yqc

        def proj1_group(XT, ykc, yqc, s, mt):
            # one k/q projection tile (N=512), fp8 DoubleRow over both k-tiles.
            ps_t = pp.tile([128, 512], F32, tag="bank", name="p1b")
            for kk in (0, 1):
                nc.tensor.matmul(
                    ps_t[:],
                    wkq[s][:, kk * 768 + mt * 128: kk * 768 + (mt + 1) * 128],
                    XT[s][:, kk * 512:(kk + 1) * 512],
                    start=(kk == 0), stop=(kk == 1),
                )
            if mt < 2:
                dest = ykc[s][:, mt * 512:(mt + 1) * 512]
            elif mt < 4:
                dest = yqc[s][:, (mt - 2) * 1024: (mt - 2) * 1024 + 512]
            else:
                dest = yqc[s][:, (mt - 4) * 1024 + 512: (mt - 4) * 1024 + 1024]
            if mt in (1, 4):
                nc.vector.tensor_copy(dest, ps_t[:])
            else:
                nc.scalar.activation(dest, ps_t[:], AF.Copy)

        def vproj_group(XT, V, s, wpair):
            # token-major V via fp8 DoubleRow; two window-pairs per bank.
            ps_t = pp.tile([128, 512], F32, tag="bank", name="vpb")
            for half in (0, 1):
                w4 = wpair * 2 + half
                for kk in (0, 1):
                    nc.tensor.matmul(
                        ps_t[:, half * 256:(half + 1) * 256],
                        XT[s][:, kk * 512 + w4 * 128: kk * 512 + (w4 + 1) * 128],
                        wv[s][:, kk * 256:(kk + 1) * 256],
                        start=(kk == 0), stop=(kk == 1),
                    )
            vt = sb.tile([128, 512], BF16, tag=f"v{s}", name=f"v{s}_{wpair}",
                         bufs=4)
            if wpair == 0:
                nc.scalar.activation(vt[:], ps_t[:], AF.Copy)
            else:
                nc.vector.tensor_copy(vt[:], ps_t[:])
            V[s][wpair] = vt

        P1L = [(s, mt) for s in (0, 1) for mt in range(6)]
        VPL = [(s, wp) for s in (0, 1) for wp in (0, 1)]

        def emit_proj_all(XT, ykc, yqc, V):
            for s, mt in P1L:
                proj1_group(XT, ykc, yqc, s, mt)
            for s, wp in VPL:
                vproj_group(XT, V, s, wp)

        def qk_stage(w4, ykc, yqc):
            # S^T in two 2-bank tiles (j-pairs); E col = j*512+s*256+hi*128+t*64+n
            bank = {}
            for b in (0, 1):
                bank[b] = pp.tile([128, 1024], F32, tag="qk", name=f"qk{b}",
                                  bufs=2)
            for j in range(4):
                bt = bank[j // 2]
                co = (j % 2) * 512
                for s in (0, 1):
                    for hi in (0, 1):
                        for p in (0, 1):
                            kT = ykc[s][j * 32:(j + 1) * 32,
                                        hi * 512 + w4 * 128 + p * 64:
                                        hi * 512 + w4 * 128 + p * 64 + 64]
                            q2 = yqc[s][j * 32:(j + 1) * 32,
                                        hi * 1024: (hi + 1) * 1024].rearrange(
                                "p (r c) -> p r c", r=2)[
                                :, :, w4 * 128 + p * 64: w4 * 128 + p * 64 + 64]
                            nc.tensor.matmul(
                                bt[p * 64:p * 64 + 64,
                                   co + s * 256 + hi * 128:
                                   co + s * 256 + (hi + 1) * 128],
                                kT, q2, start=True, stop=True,
                                tile_position=(j * 32, p * 64),
                            )
            return bank

        def exp_stage(w4, bank):
            # E[:, b*1024:(b+1)*1024] = exp(bank[b]) * ebt[:, ...]
            E = sb.tile([128, 2048], BF16, tag="eE", name="eE", bufs=3)
            e0 = sb.tile([128, 2048], BF16, tag="e0", name="e0", bufs=2)
            for b in (0, 1):
                nc.scalar.activation(
                    e0[:, b * 1024:(b + 1) * 1024], bank[b][:], AF.Exp)
            nc.vector.tensor_mul(E[:, 0:1024], e0[:, 0:1024], ebt[:, 0:1024])
            nc.gpsimd.tensor_mul(E[:, 1024:2048], e0[:, 1024:2048],
                                 ebt[:, 1024:2048])
            return E

        def av_stage(w4, E, V):
            # AV into o-banks; sums into one 2-bank tile (qk tag ring);
            # col = s*256 + hi*128 + t*64 + n, d2 col = pp*512 + that
            o_b = {}
            d2 = pp.tile([128, 1024], F32, tag="qk", name="d2", bufs=2)
            d_b = {0: d2[:, 0:512], 1: d2[:, 512:1024]}
            for pp_ in (0, 1):
                o_b[pp_] = pp.tile([128, 512], F32, tag="bank", name=f"ob{pp_}")
                # j-halves first: E[:, 0:1024] is ready before E[:, 1024:2048]
                for jh in (0, 1):
                    for s in (0, 1):
                        for hi in (0, 1):
                            for j in (2 * jh, 2 * jh + 1):
                                h = hi * 4 + j
                                ecols = E[pp_ * 64:pp_ * 64 + 64,
                                          j * 512 + s * 256 + hi * 128:
                                          j * 512 + s * 256 + (hi + 1) * 128]
                                vsl = V[s][w4 // 2][pp_ * 64:pp_ * 64 + 64,
                                                    (w4 % 2) * 256 + h * 32:
                                                    (w4 % 2) * 256 + (h + 1) * 32]
                                nc.tensor.matmul(
                                    o_b[pp_][j * 32:(j + 1) * 32,
                                             s * 256 + hi * 128:
                                             s * 256 + (hi + 1) * 128],
                                    vsl, ecols, start=True, stop=True,
                                    tile_position=(pp_ * 64, j * 32),
                                )
                # sums: one MM per j over the full 512-col row block.
                for j in range(4):
                    nc.tensor.matmul(
                        d_b[pp_][j * 32:(j + 1) * 32, :],
                        ones32[pp_ * 64:pp_ * 64 + 64, 0:32],
                        E[pp_ * 64:pp_ * 64 + 64, j * 512:(j + 1) * 512],
                        start=True, stop=True,
                        tile_position=(pp_ * 64, j * 32),
                    )
            return o_b, d2

        def norm_stage(w4, o_b, d2):
            # mirror col = s*512 + hi*256 + t*128 + pp*64 + n  (pp innermost)
            mirror = sb.tile([128, 1024], BF16, tag="mb", name="mb", bufs=3)
            rec = sb.tile([128, 1024], F32, tag="rec", name="rec", bufs=2)
            nc.vector.reciprocal_approx_fast(rec[:], d2[:])
            for pp_ in (0, 1):
                dst = mirror.rearrange(
                    "q (b pb n) -> q b pb n", pb=2, n=64)[:, :, pp_, :]
                nc.vector.tensor_mul(dst, o_b[pp_][:],
                                     rec[:, pp_ * 512:(pp_ + 1) * 512])
            return mirror

        def op_stage(ci, w4, mirror, st):
            # out-projections, token-major; shares the qk tag ring.
            ps_t = pp.tile([128, 1024], F32, tag="qk", name="opb", bufs=2)
            for oi, (nm, srcs, tt) in enumerate((
                ("x", (0, 0, 1, 1), 0),
                ("h", (1, 1, 0, 0), 1),
            )):
                for z in range(4):
                    s = srcs[z]
                    hi = z % 2
                    lhs = mirror[:, s * 512 + hi * 256 + tt * 128:
                                 s * 512 + hi * 256 + (tt + 1) * 128]
                    nc.tensor.matmul(
                        ps_t[:, oi * 256:(oi + 1) * 256],
                        lhs, wp_t[nm][:, z * 256:(z + 1) * 256],
                        start=(z == 0), stop=(z == 3),
                    )
            # pair two window-pairs per staging tile -> one DMA per output
            half = w4 % 2
            if half == 0 or st["osb"] is None:
                st["osb"] = sb.tile([128, 1024], F32, tag="osb", name="osb",
                                    bufs=2)
            o_sb = st["osb"]
            nc.scalar.activation(o_sb[:, half * 512:half * 512 + 512],
                                 ps_t[:, 0:512], AF.Copy)
            if half == 1:
                for oi, outap in enumerate((out_x, out_h)):
                    src = o_sb.rearrange("p (h o c) -> p h o c", h=2, o=2)[
                        :, :, oi, :]
                    if isinstance(ci, int):
                        dst = outap[(ci * CH + w4 - 1) * 128:
                                    (ci * CH + w4 + 1) * 128, :]
                    else:
                        dst = outap[bass.ds((ci * CH + w4 - 1) * 128, 256), :]
                    nc.sync.dma_start(
                        dst.rearrange("(h p) c -> p h c", h=2), src)
                st["osb"] = None

        def emit_chunk(ci, k, n, st):
            XT, ykc, yqc, V = st["P"]
            nxt = None
            if k + 1 < n:
                XTn = dma_xt(ci + 1)
                yk2, yq2 = alloc_proj()
                V2 = {0: [None, None], 1: [None, None]}
                nxt = (XTn, yk2, yq2, V2)
            for w4 in range(CH):
                bank = qk_stage(w4, ykc, yqc)
                # exp first so the Scalar queue frees qk banks promptly
                E = exp_stage(w4, bank)
                # spread next chunk's projections into this chunk's steps
                if nxt is not None:
                    for s, mt in P1L[3 * w4: 3 * w4 + 3]:
                        proj1_group(XTn, yk2, yq2, s, mt)
                    s, wp = VPL[w4]
                    vproj_group(XTn, V2, s, wp)
                # deferred AV of previous step
                if st["av"] is not None:
                    (pci, pw4, pE, pV) = st["av"]
                    o_b, d2 = av_stage(pw4, pE, pV)
                    mirror = norm_stage(pw4, o_b, d2)
                    st["op2"] = st["op1"]
                    st["op1"] = (pci, pw4, mirror)
                st["av"] = (ci, w4, E, V)
                # out-projection lagging two steps
                if st["op2"] is not None:
                    (oci, ow4, omir) = st["op2"]
                    op_stage(oci, ow4, omir, st)
                    st["op2"] = None
            st["P"] = nxt

        def flush(st):
            if st["av"] is not None:
                (pci, pw4, pE, pV) = st["av"]
                o_b, d2 = av_stage(pw4, pE, pV)
                mirror = norm_stage(pw4, o_b, d2)
                st["op2"] = st["op1"]
                st["op1"] = (pci, pw4, mirror)
                st["av"] = None
            for key in ("op2", "op1"):
                if st[key] is not None:
                    (oci, ow4, omir) = st[key]
                    op_stage(oci, ow4, omir, st)
                    st[key] = None

        engs = (mybir.EngineType.PE, mybir.EngineType.Activation,
                mybir.EngineType.DVE, mybir.EngineType.SP,
                mybir.EngineType.Pool)

        def unrollable(iv0, n):
            st = {"av": None, "op1": None, "op2": None, "osb": None}
            XT = dma_xt(iv0)
            dma_late_weights()
            ykc, yqc = alloc_proj()
            V = {0: [None, None], 1: [None, None]}
            emit_proj_all(XT, ykc, yqc, V)
            st["P"] = (XT, ykc, yqc, V)
            for k in range(n):
                emit_chunk(iv0 + k, k, n, st)
            flush(st)

        tc.For_i_unrolled_general(0, NCH, 1, unrollable, max_unroll=unroll,
                                  hint_engines=engs)


def prep_inputs(inputs, n_cores=8):
    import ml_dtypes
    bf = ml_dtypes.bfloat16
    f8 = ml_dtypes.float8_e4m3
    DIM, HEADS, WS = 256, 8, 8
    N = WS * WS
    B = inputs["input_x"].shape[0]
    Wc = B // n_cores
    T = Wc * N
    hd = DIM // HEADS
    scale = hd ** -0.5

    Ws_, We_ = np.asarray(inputs["Ws"]), np.asarray(inputs["We"])
    bs_, be_ = np.asarray(inputs["bs"]), np.asarray(inputs["be"])
    assert np.all(bs_ == 0) and np.all(be_ == 0), "nonzero proj1 bias unsupported"
    Wpv_, Wph_ = np.asarray(inputs["Wpv"]), np.asarray(inputs["Wph"])
    rpi = np.asarray(inputs["rpi"])

    def kq_weights(Wfull, s_v, s_h):
        k = Wfull[:, 0:256]
        qv = Wfull[:, 512:768] * s_v
        qh = Wfull[:, 768:1024] * s_h
        w = np.concatenate([k, qv, qh], axis=1)
        return np.ascontiguousarray(w.reshape(2, 128, 768)).astype(bf)

    wkq_s = kq_weights(Ws_, scale, scale * scale)
    wkq_e = kq_weights(We_, 1.0, scale)
    wv_s = np.ascontiguousarray(Ws_[:, 256:512].reshape(2, 128, 256)).astype(bf)
    wv_e = np.ascontiguousarray(We_[:, 256:512].reshape(2, 128, 256)).astype(bf)
    wpv = np.ascontiguousarray(Wpv_.reshape(4, 128, 256)).astype(bf)
    wph = np.ascontiguousarray(Wph_.reshape(4, 128, 256)).astype(bf)

    def eb_master(tabs):
        # tabs[s][t] = bias table (T, H); col = j*512+s*256+hi*128+t*64+n
        m = np.zeros((128, 2048), np.float32)
        for s in (0, 1):
            for t in (0, 1):
                tab = tabs[s][t]
                for h in range(8):
                    hi, j = divmod(h, 4)
                    b = tab[rpi.reshape(-1), h].reshape(N, N)
                    for p in (0, 1):
                        m[64 * p:64 * p + 64,
                          j * 512 + s * 256 + hi * 128 + t * 64:
                          j * 512 + s * 256 + hi * 128 + t * 64 + 64] = np.exp(b.T)
        return m

    ebc = eb_master({
        0: {0: np.asarray(inputs["tcv"]), 1: np.asarray(inputs["tsh"])},
        1: {0: np.asarray(inputs["tsv"]), 1: np.asarray(inputs["tch"])},
    }).astype(bf)

    xs_all = np.asarray(inputs["state_x"], dtype=np.float32).reshape(n_cores, T, DIM)
    xe_all = np.asarray(inputs["input_x"], dtype=np.float32).reshape(n_cores, T, DIM)

    def xt_tiles(x):
        xt = np.ascontiguousarray(x.T).astype(bf)
        return np.ascontiguousarray(
            xt.reshape(2, 128, T // 128, 128).transpose(2, 0, 1, 3))

    in_maps = []
    for c in range(n_cores):
        in_maps.append({
            "xt_s": xt_tiles(xs_all[c]), "xt_e": xt_tiles(xe_all[c]),
            "wkq_s": wkq_s, "wkq_e": wkq_e, "wv_s": wv_s, "wv_e": wv_e,
            "wpv": wpv, "wph": wph, "ebc": ebc,
        })
    return in_maps


def _get_compiled():
    key = (W_CORE, UNROLL)
    if key not in _cache:
        nc = bacc.Bacc("TRN2", target_bir_lowering=False, debug=False,
                       num_devices=N_CORES)
        with tile.TileContext(nc) as tc:
            build_kernel(nc, tc, W_CORE, unroll=UNROLL)
        nc.compile()
        _cache[key] = nc
    return _cache[key]


def kernel(**inputs):
    nc = _get_compiled()
    in_maps = prep_inputs(inputs, N_CORES)
    res = run_bass_kernel_spmd(nc, in_maps, list(range(N_CORES)), trace=False)
    bpv = np.asarray(inputs["bpv"])
    bph = np.asarray(inputs["bph"])
    B = np.asarray(inputs["input_x"]).shape[0]
    ox = np.concatenate([r["out_x"] for r in res.results], axis=0)
    oh = np.concatenate([r["out_h"] for r in res.results], axis=0)
    ox = (ox.reshape(B, NTOK, DIM) + bpv).astype(np.float32)
    oh = (oh.reshape(B, NTOK, DIM) + bph).astype(np.float32)
    return ox, oh


# revision 57
# speedup vs baseline: 1.1872x; 1.1872x over previous
"""Trainium2 Bass kernel for nn_BlockRecurrentSwinIRBlock (self-contained).

kernel(**inputs) takes the FULL unsharded inputs (B=2048 windows, 64 tokens,
C=256) and returns (output_x, state_out), each (2048, 64, 256) fp32.

Strategy: data-parallel over windows across 8 NeuronCores (256 windows/core).
All matmuls bf16 with fp32 PSUM accumulation.

v2 changes over v1 (1.71ms):
  - software pipeline restructured: out-projections lag their window-pair by
    two steps in the Tensor queue, so the Vector-side normalize chain
    (reciprocal + mul) overlaps the next pair's QK/AV instead of stalling PE.
  - j-major E layout -> exp activations write contiguous destinations.
  - d-banks use the same (s,hi,t,n) column order as o-banks and mirror keeps
    pp innermost -> normalization is 2 recips + 2 muls per pair (was 2+4).
  - elementwise work split across Scalar (exp, vproj cast), Vector (recip,
    bias-mul, one norm-mul), GpSimd (one norm-mul, proj1 copies).
  - out-projection results DMA'd straight from PSUM (no SBUF staging).
  - vproj PSUM pairs two window-pairs per bank -> half the casts.
"""
import sys
import numpy as np

for _p in ("/opt/trn_rl_repo", "/opt/trn_rl_repo/concourse"):
    if _p not in sys.path:
        sys.path.insert(0, _p)

import concourse.bass as bass
import concourse.tile as tile
from concourse import bacc, mybir
from concourse.bass_utils import run_bass_kernel_spmd
from contextlib import ExitStack

N_CORES = 8
NTOK = 64
DIM = 256
B_TOTAL = 2048
W_CORE = B_TOTAL // N_CORES
UNROLL = 32

_cache = {}

BF16 = mybir.dt.bfloat16
FP8 = mybir.dt.float8e4
F32 = mybir.dt.float32
AF = mybir.ActivationFunctionType
DR = mybir.MatmulPerfMode.DoubleRow
CH = 4          # window-pairs per chunk
# fp8 prescales: k x64, qv/qh x512, v x64 (powers of two; folded back out via
# the exp scale argument and the out-projection weights)
K_SC = 1.0
Q_SC = 1.0
V_SC = 1.0
S_SCALE = 1.0


def build_kernel(nc, tc, W, unroll=UNROLL):
    WP = W // 2
    T = W * 64
    NCH = WP // CH
    assert WP % CH == 0

    xt_s = nc.dram_tensor("xt_s", [WP, 2, 128, 128], BF16, kind="ExternalInput").ap()
    xt_e = nc.dram_tensor("xt_e", [WP, 2, 128, 128], BF16, kind="ExternalInput").ap()
    wkq_s = nc.dram_tensor("wkq_s", [2, 128, 768], BF16, kind="ExternalInput").ap()
    wkq_e = nc.dram_tensor("wkq_e", [2, 128, 768], BF16, kind="ExternalInput").ap()
    wv_s = nc.dram_tensor("wv_s", [2, 128, 256], BF16, kind="ExternalInput").ap()
    wv_e = nc.dram_tensor("wv_e", [2, 128, 256], BF16, kind="ExternalInput").ap()
    wpv = nc.dram_tensor("wpv", [4, 128, 256], BF16, kind="ExternalInput").ap()
    wph = nc.dram_tensor("wph", [4, 128, 256], BF16, kind="ExternalInput").ap()
    # combined exp(bias) master, j-major: col 1024*j? -> see eb layout below:
    # col = j*512 + s*256 + hi*128 + t*64 + n   (rows: p*64 + ktok)
    ebc = nc.dram_tensor("ebc", [128, 2048], BF16, kind="ExternalInput").ap()

    out_x = nc.dram_tensor("out_x", [T, 256], F32, kind="ExternalOutput").ap()
    out_h = nc.dram_tensor("out_h", [T, 256], F32, kind="ExternalOutput").ap()

    xt_d = {0: xt_s, 1: xt_e}
    wkq_d = {0: wkq_s, 1: wkq_e}
    wv_d = {0: wv_s, 1: wv_e}

    with ExitStack() as ctx:
        wpool = ctx.enter_context(tc.tile_pool(name="weights", bufs=1))
        sb = ctx.enter_context(tc.tile_pool(name="sb", bufs=3))
        pp = ctx.enter_context(tc.tile_pool(name="pp", bufs=4, space="PSUM"))

        wkq = {}
        wv = {}
        for s in (0, 1):
            t_ = wpool.tile([128, 2 * 768], BF16, tag=f"wkq{s}", name=f"wkq{s}t")
            for kk in (0, 1):
                nc.sync.dma_start(t_[:, kk * 768:(kk + 1) * 768], wkq_d[s][kk])
            wkq[s] = t_
            tv = wpool.tile([128, 2 * 256], BF16, tag=f"wv{s}", name=f"wv{s}t")
            for kk in (0, 1):
                nc.sync.dma_start(tv[:, kk * 256:(kk + 1) * 256], wv_d[s][kk])
            wv[s] = tv
        ebt = wpool.tile([128, 2048], BF16, tag="ebc", name="ebt")
        nc.sync.dma_start(ebt[:], ebc[:])
        wp_t = {}
        for nm, apw in (("x", wpv), ("h", wph)):
            t_ = wpool.tile([128, 4 * 256], BF16, tag=f"wp{nm}", name=f"wp{nm}t")
            for z in range(4):
                nc.sync.dma_start(t_[:, z * 256:(z + 1) * 256], apw[z])
            wp_t[nm] = t_
        ones32 = wpool.tile([128, 32], BF16, tag="ones", name="ones32")
        nc.vector.memset(ones32[:], 1.0)

        def dma_xt(ci):
            # one DMA per side: [CH, 2, 128, 128] -> [128, (kk, w4, c)]
            XT = {}
            for s in (0, 1):
                t_ = sb.tile([128, 1024], BF16, tag=f"xt{s}", name=f"xt{s}", bufs=2)
                if isinstance(ci, int):
                    src = xt_d[s][ci * CH:(ci + 1) * CH]
                else:
                    src = xt_d[s][bass.ds(ci * CH, CH)]
                for kk in (0, 1):
                    nc.sync.dma_start(
                        t_[:, kk * 512:(kk + 1) * 512].rearrange(
                            "p (w4 c) -> p w4 c", w4=CH),
                        src[:, kk].rearrange("w4 p c -> p w4 c"))
                XT[s] = t_
            return XT

        def alloc_proj():
            ykc = {}
            yqc = {}
            for s in (0, 1):
                ykc[s] = sb.tile([128, 1024], BF16, tag=f"yk{s}", name=f"yk{s}",
                                 bufs=2)
                yqc[s] = sb.tile([128, 2048], BF16, tag=f"yq{s}", name=f"yq{s}",
                                 bufs=2)
            return ykc, yqc

        def proj1_group(XT, ykc, yqc, s, mt):
            # one k/q projection tile (N=512), fp8 DoubleRow over both k-tiles.
            ps_t = pp.tile([128, 512], F32, tag="bank", name="p1b")
            for kk in (0, 1):
                nc.tensor.matmul(
                    ps_t[:],
                    wkq[s][:, kk * 768 + mt * 128: kk * 768 + (mt + 1) * 128],
                    XT[s][:, kk * 512:(kk + 1) * 512],
                    start=(kk == 0), stop=(kk == 1),
                )
            if mt < 2:
                dest = ykc[s][:, mt * 512:(mt + 1) * 512]
            elif mt < 4:
                dest = yqc[s][:, (mt - 2) * 1024: (mt - 2) * 1024 + 512]
            else:
                dest = yqc[s][:, (mt - 4) * 1024 + 512: (mt - 4) * 1024 + 1024]
            if mt in (1, 4):
                nc.vector.tensor_copy(dest, ps_t[:])
            else:
                nc.scalar.activation(dest, ps_t[:], AF.Copy)

        def vproj_group(XT, V, s, wpair):
            # token-major V via fp8 DoubleRow; two window-pairs per bank.
            ps_t = pp.tile([128, 512], F32, tag="bank", name="vpb")
            for half in (0, 1):
                w4 = wpair * 2 + half
                for kk in (0, 1):
                    nc.tensor.matmul(
                        ps_t[:, half * 256:(half + 1) * 256],
                        XT[s][:, kk * 512 + w4 * 128: kk * 512 + (w4 + 1) * 128],
                        wv[s][:, kk * 256:(kk + 1) * 256],
                        start=(kk == 0), stop=(kk == 1),
                    )
            vt = sb.tile([128, 512], BF16, tag=f"v{s}", name=f"v{s}_{wpair}",
                         bufs=4)
            nc.vector.tensor_copy(vt[:], ps_t[:])
            V[s][wpair] = vt

        P1L = [(s, mt) for s in (0, 1) for mt in range(6)]
        VPL = [(s, wp) for s in (0, 1) for wp in (0, 1)]

        def emit_proj_all(XT, ykc, yqc, V):
            for s, mt in P1L:
                proj1_group(XT, ykc, yqc, s, mt)
            for s, wp in VPL:
                vproj_group(XT, V, s, wp)

        def qk_stage(w4, ykc, yqc):
            # S^T in two 2-bank tiles (j-pairs); E col = j*512+s*256+hi*128+t*64+n
            bank = {}
            for b in (0, 1):
                bank[b] = pp.tile([128, 1024], F32, tag="qk", name=f"qk{b}",
                                  bufs=2)
            for j in range(4):
                bt = bank[j // 2]
                co = (j % 2) * 512
                for s in (0, 1):
                    for hi in (0, 1):
                        for p in (0, 1):
                            kT = ykc[s][j * 32:(j + 1) * 32,
                                        hi * 512 + w4 * 128 + p * 64:
                                        hi * 512 + w4 * 128 + p * 64 + 64]
                            q2 = yqc[s][j * 32:(j + 1) * 32,
                                        hi * 1024: (hi + 1) * 1024].rearrange(
                                "p (r c) -> p r c", r=2)[
                                :, :, w4 * 128 + p * 64: w4 * 128 + p * 64 + 64]
                            nc.tensor.matmul(
                                bt[p * 64:p * 64 + 64,
                                   co + s * 256 + hi * 128:
                                   co + s * 256 + (hi + 1) * 128],
                                kT, q2, start=True, stop=True,
                                tile_position=(j * 32, p * 64),
                            )
            return bank

        def exp_stage(w4, bank):
            # E[:, b*1024:(b+1)*1024] = exp(bank[b]) * ebt[:, ...]
            E = sb.tile([128, 2048], BF16, tag="eE", name="eE", bufs=3)
            e0 = sb.tile([128, 2048], BF16, tag="e0", name="e0", bufs=2)
            for b in (0, 1):
                nc.scalar.activation(
                    e0[:, b * 1024:(b + 1) * 1024], bank[b][:], AF.Exp)
            nc.vector.tensor_mul(E[:, 0:1024], e0[:, 0:1024], ebt[:, 0:1024])
            nc.gpsimd.tensor_mul(E[:, 1024:2048], e0[:, 1024:2048],
                                 ebt[:, 1024:2048])
            return E

        def av_stage(w4, E, V):
            # AV into o-banks; sums into one 2-bank tile (qk tag ring);
            # col = s*256 + hi*128 + t*64 + n, d2 col = pp*512 + that
            o_b = {}
            d2 = pp.tile([128, 1024], F32, tag="qk", name="d2", bufs=2)
            d_b = {0: d2[:, 0:512], 1: d2[:, 512:1024]}
            for pp_ in (0, 1):
                o_b[pp_] = pp.tile([128, 512], F32, tag="bank", name=f"ob{pp_}")
                # j-halves first: E[:, 0:1024] is ready before E[:, 1024:2048]
                for jh in (0, 1):
                    for s in (0, 1):
                        for hi in (0, 1):
                            for j in (2 * jh, 2 * jh + 1):
                                h = hi * 4 + j
                                ecols = E[pp_ * 64:pp_ * 64 + 64,
                                          j * 512 + s * 256 + hi * 128:
                                          j * 512 + s * 256 + (hi + 1) * 128]
                                vsl = V[s][w4 // 2][pp_ * 64:pp_ * 64 + 64,
                                                    (w4 % 2) * 256 + h * 32:
                                                    (w4 % 2) * 256 + (h + 1) * 32]
                                nc.tensor.matmul(
                                    o_b[pp_][j * 32:(j + 1) * 32,
                                             s * 256 + hi * 128:
                                             s * 256 + (hi + 1) * 128],
                                    vsl, ecols, start=True, stop=True,
                                    tile_position=(pp_ * 64, j * 32),
                                )
                # sums: one MM per j over the full 512-col row block.
                for j in range(4):
                    nc.tensor.matmul(
                        d_b[pp_][j * 32:(j + 1) * 32, :],
                        ones32[pp_ * 64:pp_ * 64 + 64, 0:32],
                        E[pp_ * 64:pp_ * 64 + 64, j * 512:(j + 1) * 512],
                        start=True, stop=True,
                        tile_position=(pp_ * 64, j * 32),
                    )
            return o_b, d2

        def norm_stage(w4, o_b, d2):
            # mirror col = s*512 + hi*256 + t*128 + pp*64 + n  (pp innermost)
            mirror = sb.tile([128, 1024], BF16, tag="mb", name="mb", bufs=3)
            rec = sb.tile([128, 1024], F32, tag="rec", name="rec", bufs=2)
            nc.vector.reciprocal_approx_fast(rec[:], d2[:])
            for pp_ in (0, 1):
                dst = mirror.rearrange(
                    "q (b pb n) -> q b pb n", pb=2, n=64)[:, :, pp_, :]
                nc.vector.tensor_mul(dst, o_b[pp_][:],
                                     rec[:, pp_ * 512:(pp_ + 1) * 512])
            return mirror

        def op_stage(ci, w4, mirror, st):
            # out-projections, token-major; shares the qk tag ring.
            ps_t = pp.tile([128, 1024], F32, tag="qk", name="opb", bufs=2)
            for oi, (nm, srcs, tt) in enumerate((
                ("x", (0, 0, 1, 1), 0),
                ("h", (1, 1, 0, 0), 1),
            )):
                for z in range(4):
                    s = srcs[z]
                    hi = z % 2
                    lhs = mirror[:, s * 512 + hi * 256 + tt * 128:
                                 s * 512 + hi * 256 + (tt + 1) * 128]
                    nc.tensor.matmul(
                        ps_t[:, oi * 256:(oi + 1) * 256],
                        lhs, wp_t[nm][:, z * 256:(z + 1) * 256],
                        start=(z == 0), stop=(z == 3),
                    )
            # pair two window-pairs per staging tile -> one DMA per output
            half = w4 % 2
            if half == 0 or st["osb"] is None:
                st["osb"] = sb.tile([128, 1024], F32, tag="osb", name="osb",
                                    bufs=2)
            o_sb = st["osb"]
            nc.scalar.activation(o_sb[:, half * 512:half * 512 + 512],
                                 ps_t[:, 0:512], AF.Copy)
            if half == 1:
                for oi, outap in enumerate((out_x, out_h)):
                    src = o_sb.rearrange("p (h o c) -> p h o c", h=2, o=2)[
                        :, :, oi, :]
                    if isinstance(ci, int):
                        dst = outap[(ci * CH + w4 - 1) * 128:
                                    (ci * CH + w4 + 1) * 128, :]
                    else:
                        dst = outap[bass.ds((ci * CH + w4 - 1) * 128, 256), :]
                    nc.sync.dma_start(
                        dst.rearrange("(h p) c -> p h c", h=2), src)
                st["osb"] = None

        def emit_chunk(ci, k, n, st):
            XT, ykc, yqc, V = st["P"]
            nxt = None
            if k + 1 < n:
                XTn = dma_xt(ci + 1)
                yk2, yq2 = alloc_proj()
                V2 = {0: [None, None], 1: [None, None]}
                nxt = (XTn, yk2, yq2, V2)
            for w4 in range(CH):
                bank = qk_stage(w4, ykc, yqc)
                # exp first so the Scalar queue frees qk banks promptly
                E = exp_stage(w4, bank)
                # spread next chunk's projections into this chunk's steps
                if nxt is not None:
                    for s, mt in P1L[3 * w4: 3 * w4 + 3]:
                        proj1_group(XTn, yk2, yq2, s, mt)
                    s, wp = VPL[w4]
                    vproj_group(XTn, V2, s, wp)
                # deferred AV of previous step
                if st["av"] is not None:
                    (pci, pw4, pE, pV) = st["av"]
                    o_b, d2 = av_stage(pw4, pE, pV)
                    mirror = norm_stage(pw4, o_b, d2)
                    st["op2"] = st["op1"]
                    st["op1"] = (pci, pw4, mirror)
                st["av"] = (ci, w4, E, V)
                # out-projection lagging two steps
                if st["op2"] is not None:
                    (oci, ow4, omir) = st["op2"]
                    op_stage(oci, ow4, omir, st)
                    st["op2"] = None
            st["P"] = nxt

        def flush(st):
            if st["av"] is not None:
                (pci, pw4, pE, pV) = st["av"]
                o_b, d2 = av_stage(pw4, pE, pV)
                mirror = norm_stage(pw4, o_b, d2)
                st["op2"] = st["op1"]
                st["op1"] = (pci, pw4, mirror)
                st["av"] = None
            for key in ("op2", "op1"):
                if st[key] is not None:
                    (oci, ow4, omir) = st[key]
                    op_stage(oci, ow4, omir, st)
                    st[key] = None

        engs = (mybir.EngineType.PE, mybir.EngineType.Activation,
                mybir.EngineType.DVE, mybir.EngineType.SP,
                mybir.EngineType.Pool)

        def unrollable(iv0, n):
            st = {"av": None, "op1": None, "op2": None, "osb": None}
            XT = dma_xt(iv0)
            ykc, yqc = alloc_proj()
            V = {0: [None, None], 1: [None, None]}
            emit_proj_all(XT, ykc, yqc, V)
            st["P"] = (XT, ykc, yqc, V)
            for k in range(n):
                emit_chunk(iv0 + k, k, n, st)
            flush(st)

        tc.For_i_unrolled_general(0, NCH, 1, unrollable, max_unroll=unroll,
                                  hint_engines=engs)


def prep_inputs(inputs, n_cores=8):
    import ml_dtypes
    bf = ml_dtypes.bfloat16
    f8 = ml_dtypes.float8_e4m3
    DIM, HEADS, WS = 256, 8, 8
    N = WS * WS
    B = inputs["input_x"].shape[0]
    Wc = B // n_cores
    T = Wc * N
    hd = DIM // HEADS
    scale = hd ** -0.5

    Ws_, We_ = np.asarray(inputs["Ws"]), np.asarray(inputs["We"])
    bs_, be_ = np.asarray(inputs["bs"]), np.asarray(inputs["be"])
    assert np.all(bs_ == 0) and np.all(be_ == 0), "nonzero proj1 bias unsupported"
    Wpv_, Wph_ = np.asarray(inputs["Wpv"]), np.asarray(inputs["Wph"])
    rpi = np.asarray(inputs["rpi"])

    def kq_weights(Wfull, s_v, s_h):
        k = Wfull[:, 0:256]
        qv = Wfull[:, 512:768] * s_v
        qh = Wfull[:, 768:1024] * s_h
        w = np.concatenate([k, qv, qh], axis=1)
        return np.ascontiguousarray(w.reshape(2, 128, 768)).astype(bf)

    wkq_s = kq_weights(Ws_, scale, scale * scale)
    wkq_e = kq_weights(We_, 1.0, scale)
    wv_s = np.ascontiguousarray(Ws_[:, 256:512].reshape(2, 128, 256)).astype(bf)
    wv_e = np.ascontiguousarray(We_[:, 256:512].reshape(2, 128, 256)).astype(bf)
    wpv = np.ascontiguousarray(Wpv_.reshape(4, 128, 256)).astype(bf)
    wph = np.ascontiguousarray(Wph_.reshape(4, 128, 256)).astype(bf)

    def eb_master(tabs):
        # tabs[s][t] = bias table (T, H); col = j*512+s*256+hi*128+t*64+n
        m = np.zeros((128, 2048), np.float32)
        for s in (0, 1):
            for t in (0, 1):
                tab = tabs[s][t]
                for h in range(8):
                    hi, j = divmod(h, 4)
                    b = tab[rpi.reshape(-1), h].reshape(N, N)
                    for p in (0, 1):
                        m[64 * p:64 * p + 64,
                          j * 512 + s * 256 + hi * 128 + t * 64:
                          j * 512 + s * 256 + hi * 128 + t * 64 + 64] = np.exp(b.T)
        return m

    ebc = eb_master({
        0: {0: np.asarray(inputs["tcv"]), 1: np.asarray(inputs["tsh"])},
        1: {0: np.asarray(inputs["tsv"]), 1: np.asarray(inputs["tch"])},
    }).astype(bf)

    xs_all = np.asarray(inputs["state_x"], dtype=np.float32).reshape(n_cores, T, DIM)
    xe_all = np.asarray(inputs["input_x"], dtype=np.float32).reshape(n_cores, T, DIM)

    def xt_tiles(x):
        xt = np.ascontiguousarray(x.T).astype(bf)
        return np.ascontiguousarray(
            xt.reshape(2, 128, T // 128, 128).transpose(2, 0, 1, 3))

    in_maps = []
    for c in range(n_cores):
        in_maps.append({
            "xt_s": xt_tiles(xs_all[c]), "xt_e": xt_tiles(xe_all[c]),
            "wkq_s": wkq_s, "wkq_e": wkq_e, "wv_s": wv_s, "wv_e": wv_e,
            "wpv": wpv, "wph": wph, "ebc": ebc,
        })
    return in_maps


def _get_compiled():
    key = (W_CORE, UNROLL)
    if key not in _cache:
        nc = bacc.Bacc("TRN2", target_bir_lowering=False, debug=False,
                       num_devices=N_CORES)
        with tile.TileContext(nc) as tc:
            build_kernel(nc, tc, W_CORE, unroll=UNROLL)
        nc.compile()
        _cache[key] = nc
    return _cache[key]


def kernel(**inputs):
    nc = _get_compiled()
    in_maps = prep_inputs(inputs, N_CORES)
    res = run_bass_kernel_spmd(nc, in_maps, list(range(N_CORES)), trace=False)
    bpv = np.asarray(inputs["bpv"])
    bph = np.asarray(inputs["bph"])
    B = np.asarray(inputs["input_x"]).shape[0]
    ox = np.concatenate([r["out_x"] for r in res.results], axis=0)
    oh = np.concatenate([r["out_h"] for r in res.results], axis=0)
    ox = (ox.reshape(B, NTOK, DIM) + bpv).astype(np.float32)
    oh = (oh.reshape(B, NTOK, DIM) + bph).astype(np.float32)
    return ox, oh
